# revision 3
# baseline (speedup 1.0000x reference)
"""Dihedral2Coord Trainium2 kernel.

Math: the reference applies K sequential dihedral-set steps; step k rotates
a suffix of the atom chain rigidly about the current J-K bond.  Every step's
transform is a proper rigid motion that moves all four pivot atoms of every
later step together, so the dihedral measured at application time equals the
dihedral of the ORIGINAL coordinates (dihedrals are invariant under rigid
motion).  Step k's rotation, expressed in original coordinates, is therefore
a fixed affine A_k computable from the original positions alone, and the
cumulative transform is the prefix product C_k = A_0 @ A_1 @ ... @ A_k.
The kernel:
  A) computes all K per-conformer Rodrigues affines in parallel,
  B) prefix-composes them with a blocked scan,
  C) applies C_{km(m)} to each atom run, where km(m) counts the steps whose
     mask includes atom m (verified prefix-structured on host).

Sharding: pure data parallelism over conformers N across 8 cores (SPMD).
"""

import sys

import numpy as np

try:
    import concourse.bass as bass
except ImportError:  # path in the grading container
    sys.path.insert(0, "/opt/trn_rl_repo")
    import concourse.bass as bass

import concourse.tile as tile
from concourse import mybir
from concourse.bass_utils import run_bass_kernel_spmd

f32 = mybir.dt.float32
i32 = mybir.dt.int32
Alu = mybir.AluOpType
Act = mybir.ActivationFunctionType

NCORES = 8
P = 128
TWO_PI = float(2.0 * np.pi)

_WAIT_CAP = 1  # this walrus build rejects >1 sync-wait per instruction


def _split_multi_waits(nc):
    """Split every instruction carrying >cap sync-waits into single-wait
    NoOps (same engine, immediately before, same block).  Waits are monotone
    semaphore conditions so this preserves semantics exactly."""
    n = 0
    for func in nc.m.functions:
        for bb in func.blocks:
            old = list(bb.instructions)
            if not any(
                i.sync_info is not None and len(i.sync_info.on_wait) > _WAIT_CAP
                for i in old
            ):
                continue
            new = []
            for inst in old:
                si = inst.sync_info
                if si is not None and len(si.on_wait) > _WAIT_CAP:
                    waits = list(si.on_wait)
                    head, tail = waits[:-_WAIT_CAP], waits[-_WAIT_CAP:]
                    for j in range(0, len(head), _WAIT_CAP):
                        n += 1
                        new.append(
                            mybir.InstNoOp(
                                name=f"{inst.name}_ws{j}",
                                engine=inst.engine,
                                sync_info=mybir.SyncInfo(
                                    on_wait=list(head[j : j + _WAIT_CAP]), on_update=[]
                                ),
                                bass_nofuse=True,
                            )
                        )
                    try:
                        si.on_wait[:] = tail
                    except TypeError:
                        inst.sync_info = mybir.SyncInfo(
                            on_wait=tail, on_update=list(si.on_update)
                        )
                new.append(inst)
            try:
                bb.instructions[:] = new
            except TypeError:
                bb.instructions = new
    return n


def _ap(base, offset_elems, dims):
    """Free-dim AP view into an SBUF tile AP `base` (partition dim kept).
    dims: list of [step, count] in elements of the tile's free space."""
    return bass.AP(
        tensor=base.tensor,
        offset=base.offset + offset_elems,
        ap=[list(base.ap[0])] + [list(d) for d in dims],
    )


def _dram_ap(t, offset, dims):
    return bass.AP(tensor=t.tensor, offset=offset, ap=[list(d) for d in dims])


def _analyse_mask(angles, move_mask):
    """Host-side structural analysis. Returns (km, runs): km[m] is the last
    step applied to atom m (-1 = never moved); runs are (start, len, k)."""
    K, M = move_mask.shape
    km = move_mask.astype(np.int64).sum(0) - 1
    kk = np.arange(K)[:, None]
    if not (move_mask == (kk <= km[None, :])).all():
        raise NotImplementedError("move_mask is not prefix-structured per atom")
    for k in range(K):
        for a in angles[k]:
            if not move_mask[:k, a].all():
                raise NotImplementedError("pivot atoms not rigidly co-moved")
    runs = []
    m = 0
    while m < M:
        j = m
        while j + 1 < M and km[j + 1] == km[m]:
            j += 1
        if km[m] >= 0:
            runs.append((m, j - m + 1, int(km[m])))
        m = j + 1
    return km, runs


def _build(angles, move_mask, NL, K, M):
    """Build the Bass module for one core handling NL conformers."""
    G = NL // P
    assert NL == G * P
    GK = G * K
    L = 8               # within-block scan length
    assert K % L == 0
    B = K // L          # blocks per conformer-group
    NB = G * B          # blocks over the flattened (g,k) axis

    angles = np.asarray(angles)
    arange_quads = bool((angles == np.arange(K * 4).reshape(K, 4)).all())
    km, runs = _analyse_mask(angles, move_mask)

    nc = bass.Bass()
    posT = nc.declare_dram_parameter("posT", [3, NL, M], f32, isOutput=False)
    vin = nc.declare_dram_parameter("vin", [NL, K], f32, isOutput=False)
    outT = nc.declare_dram_parameter("outT", [3, NL, M], f32, isOutput=True)

    with tile.TileContext(nc) as tc:
        with tc.tile_pool(name="main", bufs=1) as pool:
            # ---- SBUF tensors ----
            PL = pool.tile([P, 3, G, M], f32)    # planes; strides c:G*M, g:M, m:1
            OUT = pool.tile([P, 3, G, M], f32)
            V = pool.tile([P, G, K], f32)
            AT = pool.tile([P, 12, G, K], f32)   # A_k; q=4i+j, strides q:GK, g:K, k:1
            WT = pool.tile([P, 12, GK], f32)     # within-block prefixes
            CT = pool.tile([P, 12, GK], f32)     # full prefixes
            PT = pool.tile([P, 12, NB], f32)     # block products / prefixes
            ACC = pool.tile([P, 12 * max(GK, 64)], f32)
            AC2 = pool.tile([P, 12 * max(GK, 64)], f32)

            pl = PL[:, :, :, :]
            out_ap = OUT[:, :, :, :]
            at = AT[:, :, :, :]
            wt = WT[:, :, :]
            ct = CT[:, :, :]
            pt = PT[:, :, :]
            acc = ACC[:, :]
            ac2 = AC2[:, :]

            # ---- DMA in (pivot region first so stage A starts early) ----
            mp = min(int(angles.max()) + 1, M)
            nc.sync.dma_start(
                out=_ap(pl, 0, [[G * M, 3], [M, G], [1, mp]]),
                in_=_dram_ap(posT[:, :, :], 0,
                             [[M, P], [NL * M, 3], [P * M, G], [1, mp]]),
            )
            if mp < M:
                nc.sync.dma_start(
                    out=_ap(pl, mp, [[G * M, 3], [M, G], [1, M - mp]]),
                    in_=_dram_ap(posT[:, :, :], mp,
                                 [[M, P], [NL * M, 3], [P * M, G], [1, M - mp]]),
                )
            nc.sync.dma_start(
                out=V[:, :, :],
                in_=_dram_ap(vin[:, :], 0, [[K, P], [P * K, G], [1, K]]),
            )

            # ---- helpers ----
            tmp_idx = [0]

            def T(dt=f32):
                tmp_idx[0] += 1
                return pool.tile([P, G, K], dt, name=f"tmp{tmp_idx[0]}")

            def mul(a, b):
                o = T(); nc.vector.tensor_mul(o, a, b); return o

            def add(a, b):
                o = T(); nc.vector.tensor_add(o, a, b); return o

            def sub(a, b):
                o = T(); nc.vector.tensor_sub(o, a, b); return o

            def ts(a, s1, s2, op0, op1=None):
                o = T()
                if s2 is None:
                    nc.vector.tensor_scalar(o, a, s1, None, op0)
                else:
                    nc.vector.tensor_scalar(o, a, s1, s2, op0, op1)
                return o

            def activ(a, fn):
                o = T(); nc.scalar.activation(o, a, fn); return o

            def dot3(a, b):
                return add(add(mul(a[0], b[0]), mul(a[1], b[1])), mul(a[2], b[2]))

            def cross3(a, b):
                return (
                    sub(mul(a[1], b[2]), mul(a[2], b[1])),
                    sub(mul(a[2], b[0]), mul(a[0], b[2])),
                    sub(mul(a[0], b[1]), mul(a[1], b[0])),
                )

            # ---- pivot views ----
            if arange_quads:
                def piv(c, q):
                    return _ap(pl, c * G * M + q, [[M, G], [4, K]])
            else:
                PIV = pool.tile([P, 3, G, 4, K], f32)
                pv = PIV[:, :, :, :, :]
                for k in range(K):
                    for q in range(4):
                        nc.vector.tensor_copy(
                            _ap(pv, q * K + k, [[G * 4 * K, 3], [4 * K, G]]),
                            _ap(pl, int(angles[k, q]), [[G * M, 3], [M, G]]),
                        )

                def piv(c, q):
                    return _ap(pv, c * G * 4 * K + q * K, [[4 * K, G], [1, K]])

            pI = [piv(c, 0) for c in range(3)]
            pJ = [piv(c, 1) for c in range(3)]
            pK_ = [piv(c, 2) for c in range(3)]
            pLl = [piv(c, 3) for c in range(3)]

            # ---- stage A ----
            rIJ = [sub(pJ[c], pI[c]) for c in range(3)]
            rJK = [sub(pK_[c], pJ[c]) for c in range(3)]
            rKL = [sub(pLl[c], pK_[c]) for c in range(3)]
            nIJK = cross3(rIJ, rJK)
            nJKL = cross3(rJK, rKL)
            mm = cross3(nIJK, rJK)
            y0 = dot3(mm, nJKL)
            x0 = dot3(nIJK, nJKL)
            l1 = activ(dot3(nIJK, nIJK), Act.Sqrt)
            lm = activ(dot3(mm, mm), Act.Sqrt)
            jks = dot3(rJK, rJK)
            x1 = mul(x0, lm)
            y1 = mul(y0, l1)
            hs = add(mul(x1, x1), mul(y1, y1))
            hr = T(); nc.vector.reciprocal(hr, hs)
            rh = activ(hr, Act.Sqrt)            # 1/hypot
            ccur = mul(x1, rh)
            scur = mul(y1, rh)
            jkr = T(); nc.vector.reciprocal(jkr, jks)
            jrs = activ(jkr, Act.Sqrt)          # 1/|rJK|
            ax = [mul(rJK[c], jrs) for c in range(3)]

            # sin/cos of targets with range reduction (Sin table ok |x|<~3.55)
            def reduced_sin(shift_quarter, extra):
                q = ts(V[:, :, :], 1.0 / TWO_PI, 1024.0 + shift_quarter,
                       Alu.mult, Alu.add)
                qi = T(i32)
                nc.vector.tensor_copy(qi, q)     # f32->i32 rounds to nearest
                qf = T()
                nc.vector.tensor_copy(qf, qi)
                t = ts(qf, -TWO_PI, 1024.0 * TWO_PI + extra, Alu.mult, Alu.add)
                return activ(add(V[:, :, :], t), Act.Sin)

            sv = reduced_sin(0.0, 0.0)
            cv = reduced_sin(0.25, float(np.pi / 2))

            c_ = add(mul(cv, ccur), mul(sv, scur))      # cos(v - cur)
            s_ = sub(mul(sv, ccur), mul(cv, scur))      # sin(v - cur)
            t1 = ts(c_, -1.0, 1.0, Alu.mult, Alu.add)   # 1 - cos

            tax = [mul(t1, ax[c]) for c in range(3)]
            sax = [mul(s_, ax[c]) for c in range(3)]
            txy = mul(tax[0], ax[1])
            txz = mul(tax[0], ax[2])
            tyz = mul(tax[1], ax[2])

            def aq(q):
                return _ap(at, q * GK, [[K, G], [1, K]])

            for i in range(3):
                nc.vector.tensor_add(aq(4 * i + i), mul(tax[i], ax[i]), c_)
            nc.vector.tensor_sub(aq(1), txy, sax[2])
            nc.vector.tensor_add(aq(4), txy, sax[2])
            nc.vector.tensor_add(aq(2), txz, sax[1])
            nc.vector.tensor_sub(aq(8), txz, sax[1])
            nc.vector.tensor_sub(aq(6), tyz, sax[0])
            nc.vector.tensor_add(aq(9), tyz, sax[0])
            for i in range(3):
                s2 = add(add(mul(aq(4 * i + 0), pJ[0]), mul(aq(4 * i + 1), pJ[1])),
                         mul(aq(4 * i + 2), pJ[2]))
                nc.vector.tensor_sub(aq(4 * i + 3), pJ[i], s2)

            # ---- stage B: blocked prefix composition ----
            at_flat = _ap(at, 0, [[GK, 12], [1, GK]])

            def compose(dst, dq, dbd, doff, left, lq, lbd, loff,
                        right, rq, rbd, roff):
                """dst[i,j,*] = sum_m left[i,m,*]*right[m,j,*]; dst[i,3,*] +=
                left[i,3,*].  *bd = batch [step,count] dims (equal counts)."""
                counts = [d[1] for d in dbd]
                assert [d[1] for d in lbd] == counts
                assert [d[1] for d in rbd] == counts
                nb = 1
                for cnt in counts:
                    nb *= cnt
                abd = []
                stp = 1
                for cnt in reversed(counts):
                    abd.insert(0, [stp, cnt])
                    stp *= cnt

                def accv(base):
                    return _ap(base, 0, [[4 * nb, 3], [nb, 4]] + abd)

                for mrow in range(3):
                    tgt = acc if mrow == 0 else ac2
                    nc.vector.tensor_mul(
                        accv(tgt),
                        _ap(right, roff + 4 * mrow * rq, [[0, 3], [rq, 4]] + rbd),
                        _ap(left, loff + mrow * lq, [[4 * lq, 3], [0, 4]] + lbd),
                    )
                    if mrow == 1:
                        nc.vector.tensor_add(accv(acc), accv(acc), accv(ac2))
                nc.vector.tensor_add(
                    _ap(dst, doff, [[4 * dq, 3], [dq, 4]] + dbd),
                    accv(acc), accv(ac2),
                )
                bias_d = _ap(dst, doff + 3 * dq, [[4 * dq, 3]] + dbd)
                nc.vector.tensor_add(
                    bias_d, bias_d,
                    _ap(left, loff + 3 * lq, [[4 * lq, 3]] + lbd),
                )

            # seed: W[:, 8b] = A[:, 8b]
            nc.vector.tensor_copy(
                _ap(wt, 0, [[GK, 12], [L, NB]]),
                _ap(at_flat, 0, [[GK, 12], [L, NB]]),
            )
            # within-block scan
            for t in range(1, L):
                compose(wt, GK, [[L, NB]], t,
                        wt, GK, [[L, NB]], t - 1,
                        at_flat, GK, [[L, NB]], t)
            # block products
            nc.vector.tensor_copy(
                _ap(pt, 0, [[NB, 12], [1, NB]]),
                _ap(wt, L - 1, [[GK, 12], [L, NB]]),
            )
            # per-group block-prefix chains
            for j in range(1, B):
                compose(pt, NB, [[B, G]], j,
                        pt, NB, [[B, G]], j - 1,
                        pt, NB, [[B, G]], j)
            # distribute: block 0 copies, blocks b>=1 get P[b-1] @ W
            # (per output row i: ISA allows at most 3 free dims per AP)
            nc.vector.tensor_copy(
                _ap(ct, 0, [[GK, 12], [K, G], [1, L]]),
                _ap(wt, 0, [[GK, 12], [K, G], [1, L]]),
            )
            nk = (B - 1) * L
            d_jbt = [[GK, 4], [L, B - 1], [1, L]]       # wt/ct: [j][b][t]
            d_acc = [[nk, 4], [L, B - 1], [1, L]]       # acc contiguous
            d_left = [[0, 4], [1, B - 1], [0, L]]       # pt broadcast j,t
            for g in range(G):
                for i in range(3):
                    for mrow in range(3):
                        tgt = acc if mrow == 0 else ac2
                        nc.vector.tensor_mul(
                            _ap(tgt, 0, d_acc),
                            _ap(wt, 4 * mrow * GK + g * K + L, d_jbt),
                            _ap(pt, (4 * i + mrow) * NB + g * B, d_left),
                        )
                        if mrow == 1:
                            nc.vector.tensor_add(
                                _ap(acc, 0, d_acc), _ap(acc, 0, d_acc),
                                _ap(ac2, 0, d_acc),
                            )
                    nc.vector.tensor_add(
                        _ap(ct, 4 * i * GK + g * K + L, d_jbt),
                        _ap(acc, 0, d_acc), _ap(ac2, 0, d_acc),
                    )
                    bias_d = _ap(ct, (4 * i + 3) * GK + g * K + L,
                                 [[L, B - 1], [1, L]])
                    nc.vector.tensor_add(
                        bias_d, bias_d,
                        _ap(pt, (4 * i + 3) * NB + g * B, [[1, B - 1], [0, L]]),
                    )

            # ---- stage C: apply C_{km(m)} per atom run ----
            def apply_runs(starts, length, ks):
                nr = len(starts)
                sm = (starts[1] - starts[0]) if nr > 1 else 0
                sk = (ks[1] - ks[0]) if nr > 1 else 0
                m0, k0 = starts[0], ks[0]
                d_pl = [[M, G], [sm, nr], [1, length]]
                d_c = [[K, G], [sk, nr], [0, length]]
                d_acc = [[nr * length, G], [length, nr], [1, length]]
                for i in range(3):
                    for cc in range(3):
                        tgt = acc if cc == 0 else ac2
                        nc.vector.tensor_mul(
                            _ap(tgt, 0, d_acc),
                            _ap(pl, cc * G * M + m0, d_pl),
                            _ap(ct, (4 * i + cc) * GK + k0, d_c),
                        )
                        if cc == 1:
                            nc.vector.tensor_add(
                                _ap(acc, 0, d_acc), _ap(acc, 0, d_acc),
                                _ap(ac2, 0, d_acc),
                            )
                    nc.vector.tensor_add(
                        _ap(acc, 0, d_acc), _ap(acc, 0, d_acc), _ap(ac2, 0, d_acc)
                    )
                    nc.vector.tensor_add(
                        _ap(out_ap, i * G * M + m0, d_pl),
                        _ap(acc, 0, d_acc),
                        _ap(ct, (4 * i + 3) * GK + k0, d_c),
                    )

            by_len = {}
            for (m0, ln, k) in runs:
                by_len.setdefault(ln, []).append((m0, k))
            for ln, rs in sorted(by_len.items()):
                starts = [r[0] for r in rs]
                ks = [r[1] for r in rs]
                nr = len(rs)
                regular = nr <= 2 or (
                    all(starts[r] == starts[0] + r * (starts[1] - starts[0])
                        for r in range(nr))
                    and all(ks[r] == ks[0] + r * (ks[1] - ks[0]) for r in range(nr))
                )
                if regular:
                    apply_runs(starts, ln, ks)
                else:
                    for (m0, k) in rs:
                        apply_runs([m0], ln, [k])
            # unmoved atoms
            unmoved = [m for m in range(M) if km[m] < 0]
            u0 = 0
            while u0 < len(unmoved):
                u1 = u0
                while u1 + 1 < len(unmoved) and unmoved[u1 + 1] == unmoved[u1] + 1:
                    u1 += 1
                a0, ln = unmoved[u0], u1 - u0 + 1
                nc.vector.tensor_copy(
                    _ap(out_ap, a0, [[G * M, 3], [M, G], [1, ln]]),
                    _ap(pl, a0, [[G * M, 3], [M, G], [1, ln]]),
                )
                u0 = u1 + 1

            # ---- DMA out ----
            nc.sync.dma_start(
                out=_dram_ap(outT[:, :, :], 0,
                             [[M, P], [NL * M, 3], [P * M, G], [1, M]]),
                in_=out_ap,
            )

    _split_multi_waits(nc)
    return nc


_BUILD_CACHE = {}


def kernel(input, pos, angles, move_mask):
    input = np.ascontiguousarray(np.asarray(input, dtype=np.float32))
    pos = np.ascontiguousarray(np.asarray(pos, dtype=np.float32))
    angles = np.asarray(angles)
    move_mask = np.asarray(move_mask).astype(bool)

    N, K = input.shape
    _, M, three = pos.shape
    assert three == 3
    assert N % (NCORES * P) == 0
    NL = N // NCORES

    key = (N, K, M, angles.tobytes(), move_mask.tobytes())
    nc = _BUILD_CACHE.get(key)
    if nc is None:
        nc = _build(angles, move_mask, NL, K, M)
        _BUILD_CACHE[key] = nc

    in_maps = []
    for c in range(NCORES):
        sl = slice(c * NL, (c + 1) * NL)
        in_maps.append({
            "posT": np.ascontiguousarray(pos[sl].transpose(2, 0, 1)),
            "vin": np.ascontiguousarray(input[sl]),
        })

    res = run_bass_kernel_spmd(nc, in_maps, list(range(NCORES)))

    out = np.empty((N, M, 3), dtype=np.float32)
    for c in range(NCORES):
        sl = slice(c * NL, (c + 1) * NL)
        out[sl] = res.results[c]["outT"].transpose(1, 2, 0)
    return out


# revision 6
# speedup vs baseline: 1.0073x; 1.0073x over previous
"""Dihedral2Coord Trainium2 kernel.

Math: the reference applies K sequential dihedral-set steps; step k rotates
a suffix of the atom chain rigidly about the current J-K bond.  Every step's
transform is a proper rigid motion that moves all four pivot atoms of every
later step together, so the dihedral measured at application time equals the
dihedral of the ORIGINAL coordinates (dihedrals are invariant under rigid
motion).  Step k's rotation, expressed in original coordinates, is therefore
a fixed affine A_k computable from the original positions alone, and the
cumulative transform is the prefix product C_k = A_0 @ A_1 @ ... @ A_k.
The kernel:
  A) computes all K per-conformer Rodrigues affines in parallel,
  B) prefix-composes them with a blocked scan,
  C) applies C_{km(m)} to each atom run, where km(m) counts the steps whose
     mask includes atom m (verified prefix-structured on host).

Sharding: pure data parallelism over conformers N across 8 cores (SPMD).
"""

import sys

import numpy as np

try:
    import concourse.bass as bass
except ImportError:  # path in the grading container
    sys.path.insert(0, "/opt/trn_rl_repo")
    import concourse.bass as bass

import concourse.tile as tile
from concourse import mybir
from concourse.bass_utils import run_bass_kernel_spmd

f32 = mybir.dt.float32
i32 = mybir.dt.int32
Alu = mybir.AluOpType
Act = mybir.ActivationFunctionType

NCORES = 8
P = 128
TWO_PI = float(2.0 * np.pi)

_WAIT_CAP = 1  # this walrus build rejects >1 sync-wait per instruction

_HALF_PI = float(np.pi / 2)


def _register_const(nc, value, dtype=f32):
    """Mimic Bass.__init__'s register_const_ap for activation bias values."""
    if (dtype, value) in nc.const_aps.aps:
        return
    t = nc.alloc_sbuf_tensor(f"const-{dtype.name}-{value}", [128, 1], dtype)
    nc.gpsimd.memset(t.ap(), value)
    nc.const_aps.aps[(dtype, value)] = t.ap()


def _split_multi_waits(nc):
    """Split every instruction carrying >cap sync-waits into single-wait
    NoOps (same engine, immediately before, same block).  Waits are monotone
    semaphore conditions so this preserves semantics exactly."""
    n = 0
    for func in nc.m.functions:
        for bb in func.blocks:
            old = list(bb.instructions)
            if not any(
                i.sync_info is not None and len(i.sync_info.on_wait) > _WAIT_CAP
                for i in old
            ):
                continue
            new = []
            for inst in old:
                si = inst.sync_info
                if si is not None and len(si.on_wait) > _WAIT_CAP:
                    waits = list(si.on_wait)
                    head, tail = waits[:-_WAIT_CAP], waits[-_WAIT_CAP:]
                    for j in range(0, len(head), _WAIT_CAP):
                        n += 1
                        new.append(
                            mybir.InstNoOp(
                                name=f"{inst.name}_ws{j}",
                                engine=inst.engine,
                                sync_info=mybir.SyncInfo(
                                    on_wait=list(head[j : j + _WAIT_CAP]), on_update=[]
                                ),
                                bass_nofuse=True,
                            )
                        )
                    try:
                        si.on_wait[:] = tail
                    except TypeError:
                        inst.sync_info = mybir.SyncInfo(
                            on_wait=tail, on_update=list(si.on_update)
                        )
                new.append(inst)
            try:
                bb.instructions[:] = new
            except TypeError:
                bb.instructions = new
    return n


def _ap(base, offset_elems, dims):
    """Free-dim AP view into an SBUF tile AP `base` (partition dim kept).
    dims: list of [step, count] in elements of the tile's free space."""
    return bass.AP(
        tensor=base.tensor,
        offset=base.offset + offset_elems,
        ap=[list(base.ap[0])] + [list(d) for d in dims],
    )


def _dram_ap(t, offset, dims):
    return bass.AP(tensor=t.tensor, offset=offset, ap=[list(d) for d in dims])


def _analyse_mask(angles, move_mask):
    """Host-side structural analysis. Returns (km, runs): km[m] is the last
    step applied to atom m (-1 = never moved); runs are (start, len, k)."""
    K, M = move_mask.shape
    km = move_mask.astype(np.int64).sum(0) - 1
    kk = np.arange(K)[:, None]
    if not (move_mask == (kk <= km[None, :])).all():
        raise NotImplementedError("move_mask is not prefix-structured per atom")
    for k in range(K):
        for a in angles[k]:
            if not move_mask[:k, a].all():
                raise NotImplementedError("pivot atoms not rigidly co-moved")
    runs = []
    m = 0
    while m < M:
        j = m
        while j + 1 < M and km[j + 1] == km[m]:
            j += 1
        if km[m] >= 0:
            runs.append((m, j - m + 1, int(km[m])))
        m = j + 1
    return km, runs


def _build(angles, move_mask, NL, K, M):
    """Build the Bass module for one core handling NL conformers."""
    G = NL // P
    assert NL == G * P
    GK = G * K
    L = 8               # within-block scan length
    assert K % L == 0
    B = K // L          # blocks per conformer-group
    NB = G * B          # blocks over the flattened (g,k) axis

    angles = np.asarray(angles)
    arange_quads = bool((angles == np.arange(K * 4).reshape(K, 4)).all())
    km, runs = _analyse_mask(angles, move_mask)

    nc = bass.Bass()
    for cval in (1024.0, 1024.25, 1024.0 * TWO_PI, 1024.0 * TWO_PI + _HALF_PI):
        _register_const(nc, float(cval))
    nc.all_engine_barrier()
    posT = nc.declare_dram_parameter("posT", [3, NL, M], f32, isOutput=False)
    vin = nc.declare_dram_parameter("vin", [NL, K], f32, isOutput=False)
    outT = nc.declare_dram_parameter("outT", [3, NL, M], f32, isOutput=True)

    with tile.TileContext(nc) as tc:
        with tc.tile_pool(name="main", bufs=1) as pool:
            # ---- SBUF tensors ----
            PL = pool.tile([P, 3, G, M], f32)    # planes; strides c:G*M, g:M, m:1
            OUT = pool.tile([P, 3, G, M], f32)
            V = pool.tile([P, G, K], f32)
            AT = pool.tile([P, 12, G, K], f32)   # A_k; q=4i+j, strides q:GK, g:K, k:1
            WT = pool.tile([P, 12, GK], f32)     # within-block prefixes
            CT = pool.tile([P, 12, GK], f32)     # full prefixes
            PT = pool.tile([P, 12, NB], f32)     # block products / prefixes
            ACC = pool.tile([P, 12 * max(GK, 64)], f32)
            AC2 = pool.tile([P, 12 * max(GK, 64)], f32)

            pl = PL[:, :, :, :]
            out_ap = OUT[:, :, :, :]
            at = AT[:, :, :, :]
            wt = WT[:, :, :]
            ct = CT[:, :, :]
            pt = PT[:, :, :]
            acc = ACC[:, :]
            ac2 = AC2[:, :]

            # ---- DMA in (vin first: the sin/cos chain only needs V) ----
            nc.sync.dma_start(
                out=V[:, :, :],
                in_=_dram_ap(vin[:, :], 0, [[K, P], [P * K, G], [1, K]]),
            )
            nc.sync.dma_start(
                out=_ap(pl, 0, [[G * M, 3], [M, G], [1, M]]),
                in_=_dram_ap(posT[:, :, :], 0,
                             [[M, P], [NL * M, 3], [P * M, G], [1, M]]),
            )

            # ---- helpers ----
            tmp_idx = [0]

            def T(dt=f32):
                tmp_idx[0] += 1
                return pool.tile([P, G, K], dt, name=f"tmp{tmp_idx[0]}")

            def mul(a, b):
                o = T(); nc.vector.tensor_mul(o, a, b); return o

            def pmul(a, b):
                o = T(); nc.gpsimd.tensor_mul(o, a, b); return o

            def add(a, b):
                o = T(); nc.vector.tensor_add(o, a, b); return o

            def sub(a, b):
                o = T(); nc.vector.tensor_sub(o, a, b); return o

            def psub(a, b):
                o = T(); nc.gpsimd.tensor_sub(o, a, b); return o

            def aff(a, scale, bias):
                o = T()
                nc.scalar.activation(o, a, Act.Identity, bias=bias, scale=scale)
                return o

            def ts(a, s1, s2, op0, op1=None):
                o = T()
                if s2 is None:
                    nc.vector.tensor_scalar(o, a, s1, None, op0)
                else:
                    nc.vector.tensor_scalar(o, a, s1, s2, op0, op1)
                return o

            def activ(a, fn):
                o = T(); nc.scalar.activation(o, a, fn); return o

            def dot3(a, b):
                return add(add(mul(a[0], b[0]), pmul(a[1], b[1])), mul(a[2], b[2]))

            def cross3(a, b):
                return (
                    sub(mul(a[1], b[2]), pmul(a[2], b[1])),
                    sub(mul(a[2], b[0]), pmul(a[0], b[2])),
                    sub(mul(a[0], b[1]), pmul(a[1], b[0])),
                )

            # ---- pivot views ----
            if arange_quads:
                def piv(c, q):
                    return _ap(pl, c * G * M + q, [[M, G], [4, K]])
            else:
                PIV = pool.tile([P, 3, G, 4, K], f32)
                pv = PIV[:, :, :, :, :]
                for k in range(K):
                    for q in range(4):
                        nc.vector.tensor_copy(
                            _ap(pv, q * K + k, [[G * 4 * K, 3], [4 * K, G]]),
                            _ap(pl, int(angles[k, q]), [[G * M, 3], [M, G]]),
                        )

                def piv(c, q):
                    return _ap(pv, c * G * 4 * K + q * K, [[4 * K, G], [1, K]])

            pI = [piv(c, 0) for c in range(3)]
            pJ = [piv(c, 1) for c in range(3)]
            pK_ = [piv(c, 2) for c in range(3)]
            pLl = [piv(c, 3) for c in range(3)]

            # ---- stage A ----
            rIJ = [psub(pJ[c], pI[c]) for c in range(3)]
            rJK = [sub(pK_[c], pJ[c]) for c in range(3)]
            rKL = [psub(pLl[c], pK_[c]) for c in range(3)]
            nIJK = cross3(rIJ, rJK)
            nJKL = cross3(rJK, rKL)
            mm = cross3(nIJK, rJK)
            y0 = dot3(mm, nJKL)
            x0 = dot3(nIJK, nJKL)
            l1 = activ(dot3(nIJK, nIJK), Act.Sqrt)
            lm = activ(dot3(mm, mm), Act.Sqrt)
            jks = dot3(rJK, rJK)
            x1 = mul(x0, lm)
            y1 = mul(y0, l1)
            hs = add(mul(x1, x1), mul(y1, y1))
            hr = T(); nc.vector.reciprocal(hr, hs)
            rh = activ(hr, Act.Sqrt)            # 1/hypot
            ccur = mul(x1, rh)
            scur = mul(y1, rh)
            jkr = T(); nc.vector.reciprocal(jkr, jks)
            jrs = activ(jkr, Act.Sqrt)          # 1/|rJK|
            ax = [mul(rJK[c], jrs) for c in range(3)]

            # sin/cos of targets with range reduction (Sin table ok |x|<~3.55)
            def reduced_sin(shift_quarter, extra):
                q = aff(V[:, :, :], 1.0 / TWO_PI, 1024.0 + shift_quarter)
                qi = T(i32)
                nc.vector.tensor_copy(qi, q)     # f32->i32 rounds to nearest
                qf = T()
                nc.vector.tensor_copy(qf, qi)
                t = aff(qf, -TWO_PI, 1024.0 * TWO_PI + extra)
                return activ(add(V[:, :, :], t), Act.Sin)

            sv = reduced_sin(0.0, 0.0)
            cv = reduced_sin(0.25, float(np.pi / 2))

            c_ = add(mul(cv, ccur), mul(sv, scur))      # cos(v - cur)
            s_ = sub(mul(sv, ccur), mul(cv, scur))      # sin(v - cur)
            t1 = aff(c_, -1.0, 1.0)                      # 1 - cos

            tax = [mul(t1, ax[c]) for c in range(3)]
            sax = [pmul(s_, ax[c]) for c in range(3)]
            txy = mul(tax[0], ax[1])
            txz = mul(tax[0], ax[2])
            tyz = mul(tax[1], ax[2])

            def aq(q):
                return _ap(at, q * GK, [[K, G], [1, K]])

            for i in range(3):
                nc.vector.tensor_add(aq(4 * i + i), mul(tax[i], ax[i]), c_)
            nc.vector.tensor_sub(aq(1), txy, sax[2])
            nc.vector.tensor_add(aq(4), txy, sax[2])
            nc.vector.tensor_add(aq(2), txz, sax[1])
            nc.vector.tensor_sub(aq(8), txz, sax[1])
            nc.vector.tensor_sub(aq(6), tyz, sax[0])
            nc.vector.tensor_add(aq(9), tyz, sax[0])
            for i in range(3):
                s2 = add(add(mul(aq(4 * i + 0), pJ[0]), pmul(aq(4 * i + 1), pJ[1])),
                         mul(aq(4 * i + 2), pJ[2]))
                nc.vector.tensor_sub(aq(4 * i + 3), pJ[i], s2)

            # ---- stage B: blocked prefix composition ----
            at_flat = _ap(at, 0, [[GK, 12], [1, GK]])

            def compose(dst, dq, dbd, doff, left, lq, lbd, loff,
                        right, rq, rbd, roff):
                """dst[i,j,*] = sum_m left[i,m,*]*right[m,j,*]; dst[i,3,*] +=
                left[i,3,*].  *bd = batch [step,count] dims (equal counts)."""
                counts = [d[1] for d in dbd]
                assert [d[1] for d in lbd] == counts
                assert [d[1] for d in rbd] == counts
                nb = 1
                for cnt in counts:
                    nb *= cnt
                abd = []
                stp = 1
                for cnt in reversed(counts):
                    abd.insert(0, [stp, cnt])
                    stp *= cnt

                def accv(base):
                    return _ap(base, 0, [[4 * nb, 3], [nb, 4]] + abd)

                for mrow in range(3):
                    tgt = acc if mrow == 0 else ac2
                    eng = nc.gpsimd if mrow == 1 else nc.vector
                    eng.tensor_mul(
                        accv(tgt),
                        _ap(right, roff + 4 * mrow * rq, [[0, 3], [rq, 4]] + rbd),
                        _ap(left, loff + mrow * lq, [[4 * lq, 3], [0, 4]] + lbd),
                    )
                    if mrow == 1:
                        nc.vector.tensor_add(accv(acc), accv(acc), accv(ac2))
                nc.vector.tensor_add(
                    _ap(dst, doff, [[4 * dq, 3], [dq, 4]] + dbd),
                    accv(acc), accv(ac2),
                )
                bias_d = _ap(dst, doff + 3 * dq, [[4 * dq, 3]] + dbd)
                nc.vector.tensor_add(
                    bias_d, bias_d,
                    _ap(left, loff + 3 * lq, [[4 * lq, 3]] + lbd),
                )

            # seed: W[:, 8b] = A[:, 8b]
            nc.vector.tensor_copy(
                _ap(wt, 0, [[GK, 12], [L, NB]]),
                _ap(at_flat, 0, [[GK, 12], [L, NB]]),
            )
            # within-block scan
            for t in range(1, L):
                compose(wt, GK, [[L, NB]], t,
                        wt, GK, [[L, NB]], t - 1,
                        at_flat, GK, [[L, NB]], t)
            # block products
            nc.vector.tensor_copy(
                _ap(pt, 0, [[NB, 12], [1, NB]]),
                _ap(wt, L - 1, [[GK, 12], [L, NB]]),
            )
            # per-group block-prefix chains
            for j in range(1, B):
                compose(pt, NB, [[B, G]], j,
                        pt, NB, [[B, G]], j - 1,
                        pt, NB, [[B, G]], j)
            # distribute: block 0 copies, blocks b>=1 get P[b-1] @ W
            # (per output row i: ISA allows at most 3 free dims per AP)
            nc.vector.tensor_copy(
                _ap(ct, 0, [[GK, 12], [K, G], [1, L]]),
                _ap(wt, 0, [[GK, 12], [K, G], [1, L]]),
            )
            nk = (B - 1) * L
            d_jbt = [[GK, 4], [L, B - 1], [1, L]]       # wt/ct: [j][b][t]
            d_acc = [[nk, 4], [L, B - 1], [1, L]]       # acc contiguous
            d_left = [[0, 4], [1, B - 1], [0, L]]       # pt broadcast j,t
            for g in range(G):
                for i in range(3):
                    for mrow in range(3):
                        tgt = acc if mrow == 0 else ac2
                        eng = nc.gpsimd if mrow == 1 else nc.vector
                        eng.tensor_mul(
                            _ap(tgt, 0, d_acc),
                            _ap(wt, 4 * mrow * GK + g * K + L, d_jbt),
                            _ap(pt, (4 * i + mrow) * NB + g * B, d_left),
                        )
                        if mrow == 1:
                            nc.vector.tensor_add(
                                _ap(acc, 0, d_acc), _ap(acc, 0, d_acc),
                                _ap(ac2, 0, d_acc),
                            )
                    nc.vector.tensor_add(
                        _ap(ct, 4 * i * GK + g * K + L, d_jbt),
                        _ap(acc, 0, d_acc), _ap(ac2, 0, d_acc),
                    )
                    bias_d = _ap(ct, (4 * i + 3) * GK + g * K + L,
                                 [[L, B - 1], [1, L]])
                    nc.vector.tensor_add(
                        bias_d, bias_d,
                        _ap(pt, (4 * i + 3) * NB + g * B, [[1, B - 1], [0, L]]),
                    )

            # ---- stage C: apply C_{km(m)} per atom run ----
            def apply_runs(starts, length, ks):
                nr = len(starts)
                sm = (starts[1] - starts[0]) if nr > 1 else 0
                sk = (ks[1] - ks[0]) if nr > 1 else 0
                m0, k0 = starts[0], ks[0]
                d_pl = [[M, G], [sm, nr], [1, length]]
                d_c = [[K, G], [sk, nr], [0, length]]
                d_acc = [[nr * length, G], [length, nr], [1, length]]
                if nr == 1:
                    # single run: coefficients are per-(partition,g) scalars;
                    # run the 9 muls on ACT (per-partition scale) per group,
                    # freeing the vector engine.
                    tmp_idx[0] += 1
                    prod = [[pool.tile([P, G * length], f32,
                                       name=f"prod{tmp_idx[0]}_{i}_{cc}")[:, :]
                             for cc in range(3)] for i in range(3)]
                    for i in range(3):
                        for cc in range(3):
                            for g in range(G):
                                nc.scalar.activation(
                                    _ap(prod[i][cc], g * length, [[1, length]]),
                                    _ap(pl, cc * G * M + g * M + m0, [[1, length]]),
                                    Act.Identity,
                                    scale=_ap(ct, (4 * i + cc) * GK + g * K + k0, [[1, 1]]),
                                )
                    for i in range(3):
                        d_t = [[length, G], [1, length]]
                        s1 = _ap(prod[i][0], 0, d_t)
                        nc.vector.tensor_add(s1, s1, _ap(prod[i][1], 0, d_t))
                        nc.vector.tensor_add(s1, s1, _ap(prod[i][2], 0, d_t))
                        for g in range(G):
                            nc.vector.tensor_scalar(
                                _ap(out_ap, i * G * M + g * M + m0, [[1, length]]),
                                _ap(prod[i][0], g * length, [[1, length]]),
                                _ap(ct, (4 * i + 3) * GK + g * K + k0, [[1, 1]]),
                                None, Alu.add,
                            )
                    return
                for i in range(3):
                    for cc in range(3):
                        tgt = acc if cc == 0 else ac2
                        eng = nc.gpsimd if cc == 1 else nc.vector
                        eng.tensor_mul(
                            _ap(tgt, 0, d_acc),
                            _ap(pl, cc * G * M + m0, d_pl),
                            _ap(ct, (4 * i + cc) * GK + k0, d_c),
                        )
                        if cc == 1:
                            nc.vector.tensor_add(
                                _ap(acc, 0, d_acc), _ap(acc, 0, d_acc),
                                _ap(ac2, 0, d_acc),
                            )
                    nc.vector.tensor_add(
                        _ap(acc, 0, d_acc), _ap(acc, 0, d_acc), _ap(ac2, 0, d_acc)
                    )
                    nc.vector.tensor_add(
                        _ap(out_ap, i * G * M + m0, d_pl),
                        _ap(acc, 0, d_acc),
                        _ap(ct, (4 * i + 3) * GK + k0, d_c),
                    )

            def dma_out_cols(a0, ln):
                nc.sync.dma_start(
                    out=_dram_ap(outT[:, :, :], a0,
                                 [[M, P], [NL * M, 3], [P * M, G], [1, ln]]),
                    in_=_ap(out_ap, a0, [[G * M, 3], [M, G], [1, ln]]),
                )

            # unmoved atoms first (ready as soon as PL lands)
            unmoved = [m for m in range(M) if km[m] < 0]
            u0 = 0
            while u0 < len(unmoved):
                u1 = u0
                while u1 + 1 < len(unmoved) and unmoved[u1 + 1] == unmoved[u1] + 1:
                    u1 += 1
                a0, ln = unmoved[u0], u1 - u0 + 1
                nc.vector.tensor_copy(
                    _ap(out_ap, a0, [[G * M, 3], [M, G], [1, ln]]),
                    _ap(pl, a0, [[G * M, 3], [M, G], [1, ln]]),
                )
                dma_out_cols(a0, ln)
                u0 = u1 + 1

            by_len = {}
            for (m0, ln, k) in runs:
                by_len.setdefault(ln, []).append((m0, k))
            for ln, rs in sorted(by_len.items()):
                starts = [r[0] for r in rs]
                ks = [r[1] for r in rs]
                nr = len(rs)
                regular = nr <= 2 or (
                    all(starts[r] == starts[0] + r * (starts[1] - starts[0])
                        for r in range(nr))
                    and all(ks[r] == ks[0] + r * (ks[1] - ks[0]) for r in range(nr))
                )
                if regular:
                    apply_runs(starts, ln, ks)
                else:
                    for (m0, k) in rs:
                        apply_runs([m0], ln, [k])
                # DMA this class's contiguous column span as soon as written
                lo = min(starts)
                hi = max(s + ln for s in starts)
                dma_out_cols(lo, hi - lo)

    _split_multi_waits(nc)
    return nc


_BUILD_CACHE = {}


def kernel(input, pos, angles, move_mask):
    input = np.ascontiguousarray(np.asarray(input, dtype=np.float32))
    pos = np.ascontiguousarray(np.asarray(pos, dtype=np.float32))
    angles = np.asarray(angles)
    move_mask = np.asarray(move_mask).astype(bool)

    N, K = input.shape
    _, M, three = pos.shape
    assert three == 3
    assert N % (NCORES * P) == 0
    NL = N // NCORES

    key = (N, K, M, angles.tobytes(), move_mask.tobytes())
    nc = _BUILD_CACHE.get(key)
    if nc is None:
        nc = _build(angles, move_mask, NL, K, M)
        _BUILD_CACHE[key] = nc

    in_maps = []
    for c in range(NCORES):
        sl = slice(c * NL, (c + 1) * NL)
        in_maps.append({
            "posT": np.ascontiguousarray(pos[sl].transpose(2, 0, 1)),
            "vin": np.ascontiguousarray(input[sl]),
        })

    res = run_bass_kernel_spmd(nc, in_maps, list(range(NCORES)))

    out = np.empty((N, M, 3), dtype=np.float32)
    for c in range(NCORES):
        sl = slice(c * NL, (c + 1) * NL)
        out[sl] = res.results[c]["outT"].transpose(1, 2, 0)
    return out
